# revision 11
# baseline (speedup 1.0000x reference)
"""Trainium2 Bass kernel for a 2-layer GATv2 (DependencyGraphAnalyzer).

Strategy (8 cores, SPMD, bf16):
  - Host sorts edges by dst and shards them by dst-node range: core c owns
    nodes [c*2500, (c+1)*2500) and every edge pointing into that range.
    Softmax segments (per-dst) are therefore entirely core-local.
  - Everything feature-sized runs in bf16 (validated ~1.3e-3 end-to-end
    rel err vs the 2e-2 gate): 4x faster matmuls, faster DVE, half the
    gather bytes.
  - Each core computes the full source transform xl = x@Wl+bl into an HBM
    bf16 table and its own range's xr = x@Wr+br into SBUF. Dense-phase
    DMAs are batched 4 node-tiles per transfer (sync-engine dispatch is
    ~700ns per dma_start).
  - The per-tile one-hot scatter matrices (oh and its transpose) depend
    only on the static dst pattern: the host precomputes them as bf16
    tables, streamed per 12-tile chunk (saves 2 DVE builds per tile).
  - Edge phase runs per 128-edge tile with a 2-tile software pipeline:
    dma_gather of xl[src] rows in PREPARE_ONLY mode (desc-gen only on
    gpsimd, transfers fly on 4 rotating SWDGE queues), one-hot scatter
    matmuls accumulated in PSUM (s = xr[dst] + xl[src] via ohT- and
    identity-matmuls), Prelu on the scalar engine, att-dot on DVE, exp on
    scalar, per-head msg scaling on DVE, and segment-sum (denominator +
    message) matmuls into PSUM.
  - Softmax denominator applied after aggregation (constant per segment);
    max-subtraction skipped (scores are O(1); exp is safe in fp32 range).
  - ELU's "-1" is folded into the layer-2 biases host-side.
  - Two launches; host all-gathers the hidden state h between the layers.
"""

import numpy as np
import ml_dtypes

BF16 = ml_dtypes.bfloat16

# Problem constants (hardcoded; kernel.py must be self-contained).
N_NODES = 20000
N_EDGES = 320000
IN_DIM = 256
HID = 128
HEADS = 4
NEG_SLOPE = 0.2
NCORES = 8
NPC = N_NODES // NCORES  # 2500 own nodes per core
P = 128
GCH = 12                 # gather chunk: tiles per dma_gather call
NQ = 4                   # SWDGE queues for gathers
DB = 4                   # dense phase: node tiles per batched DMA
PREP_GATHER = True       # prepare_only + trigger_dma gather pipelining


# ---------------------------------------------------------------------------
# Host-side edge preprocessing
# ---------------------------------------------------------------------------

def prep_edges(edge_index, n_nodes=N_NODES, ncores=NCORES):
    """Sort edges by dst, shard by dst range, pad each (core, block) segment
    to a common per-block tile count, and build the device index arrays.

    Returns (Tb, per_core) where Tb[b] is the number of 128-edge tiles of
    block b (shared by all cores) and per_core[c] is a dict with:
      idx : [128, TT*8] int16  wrapped dma_gather indices (pad -> n_nodes)
      oh  : [128, TT*128] bf16 one-hot; oh[e, g*128+j] = dst(g,e) == j
      ohT : [128, TT*128] bf16 transposed; ohT[j, g*128+e] = dst(g,e) == j
    """
    npc = n_nodes // ncores
    nb = (npc + P - 1) // P
    src = np.asarray(edge_index[0], dtype=np.int64)
    dst = np.asarray(edge_index[1], dtype=np.int64)
    order = np.argsort(dst, kind="stable")
    src_s, dst_s = src[order], dst[order]

    core_of = dst_s // npc
    blk_of = core_of * nb + (dst_s - core_of * npc) // P
    counts = np.bincount(blk_of, minlength=ncores * nb).reshape(ncores, nb)
    ends = np.cumsum(counts.reshape(-1)).reshape(ncores, nb)
    starts = ends - counts

    tiles = (counts + P - 1) // P            # [ncores, nb]
    Tb = np.maximum(tiles.max(axis=0), 1)    # shared per-block tile count
    TT = int(Tb.sum())
    offs = np.concatenate([[0], np.cumsum(Tb)[:-1]])  # tile offset per block

    jj = np.arange(P, dtype=np.int32)
    per_core = []
    for c in range(ncores):
        idx_flat = np.full(TT * P, n_nodes, dtype=np.int64)  # pad -> zero row
        dloc_flat = np.full(TT * P, -1, dtype=np.int32)      # pad -> -1
        for b in range(nb):
            s, e = starts[c, b], ends[c, b]
            o = offs[b] * P
            idx_flat[o:o + e - s] = src_s[s:e]
            dloc_flat[o:o + e - s] = (dst_s[s:e] - c * npc - b * P)
        # Wrap gather indices: within a gather call over a 128-aligned
        # sub-range, index j lives at [j % 16, j // 16], replicated to 128
        # partitions. The wrap is position-linear, so any 128-aligned
        # sub-range of the stream can be sliced out for one call.
        idx_w = np.tile(idx_flat.astype(np.int16).reshape(TT * 8, 16).T, (8, 1))
        dloc = dloc_flat.reshape(TT, P)      # [tile, edge lane]
        oh = (dloc[:, :, None] == jj[None, None, :])            # [g, e, j]
        oh_h = np.ascontiguousarray(
            oh.transpose(1, 0, 2).reshape(P, TT * P)).astype(BF16)
        ohT_h = np.ascontiguousarray(
            oh.transpose(2, 0, 1).reshape(P, TT * P)).astype(BF16)
        per_core.append({"idx": np.ascontiguousarray(idx_w),
                         "oh": oh_h, "ohT": ohT_h})
    return [int(t) for t in Tb], per_core


# ---------------------------------------------------------------------------
# Bass program builder (one GATv2 layer, optionally + heads)
# ---------------------------------------------------------------------------

def build_layer(nn, npc, in_dim, heads, Tb, elu, heads_out, num_devices=NCORES):
    import concourse.bacc as bacc
    import concourse.tile as tile
    import concourse.mybir as mybir
    from contextlib import ExitStack

    f32 = mybir.dt.float32
    bf = mybir.dt.bfloat16
    AF = mybir.ActivationFunctionType
    ALU = mybir.AluOpType
    AX = mybir.AxisListType

    C = 128
    H = heads
    F = H * C
    KC = in_dim // P
    NB = len(Tb)
    TT = sum(Tb)
    blk_rows = [min(P, npc - b * P) for b in range(NB)]
    n_node_tiles = (nn + P - 1) // P

    nc = bacc.Bacc("TRN2", target_bir_lowering=False, debug=False,
                   num_devices=num_devices, num_swdge_queues=NQ)

    # --- DRAM tensors ---
    xT = nc.dram_tensor("xT", [in_dim, nn], bf, kind="ExternalInput").ap()
    xoT = nc.dram_tensor("xoT", [in_dim, npc], bf, kind="ExternalInput").ap()
    wl = nc.dram_tensor("wl", [in_dim, F], bf, kind="ExternalInput").ap()
    wr = nc.dram_tensor("wr", [in_dim, F], bf, kind="ExternalInput").ap()
    blb = nc.dram_tensor("blb", [P, F], bf, kind="ExternalInput").ap()
    brb = nc.dram_tensor("brb", [P, F], bf, kind="ExternalInput").ap()
    attb = nc.dram_tensor("attb", [P, F], bf, kind="ExternalInput").ap()
    biasb = nc.dram_tensor("biasb", [P, F], bf, kind="ExternalInput").ap()
    idx_d = nc.dram_tensor("idx", [P, TT * 8], mybir.dt.int16,
                           kind="ExternalInput").ap()
    oh_d = nc.dram_tensor("oh", [P, TT * P], bf, kind="ExternalInput").ap()
    ohT_d = nc.dram_tensor("ohT", [P, TT * P], bf, kind="ExternalInput").ap()
    ident_d = nc.dram_tensor("ident", [P, P], bf, kind="ExternalInput").ap()
    if heads_out:
        headw = nc.dram_tensor("headw", [P, 2 * C], bf, kind="ExternalInput").ap()
        headb = nc.dram_tensor("headb", [P, 2], f32, kind="ExternalInput").ap()
        an_d = nc.dram_tensor("an", [npc, 1], f32, kind="ExternalOutput").ap()
        rc_d = nc.dram_tensor("rc", [npc, 1], f32, kind="ExternalOutput").ap()
    else:
        h_d = nc.dram_tensor("h_own", [npc, F], bf, kind="ExternalOutput").ap()
    xl_full = nc.dram_tensor("xl_full", [nn + 1, F], bf, kind="Internal").ap()

    with tile.TileContext(nc) as tc, ExitStack() as es:
        cp = es.enter_context(tc.tile_pool(name="const", bufs=1))

        wl_sb = cp.tile([P, KC, F], bf, tag="wl")
        nc.sync.dma_start(wl_sb[:], wl.rearrange("(c k) f -> k c f", k=P))
        wr_sb = cp.tile([P, KC, F], bf, tag="wr")
        nc.sync.dma_start(wr_sb[:], wr.rearrange("(c k) f -> k c f", k=P))
        blb_sb = cp.tile([P, F], bf, tag="blb")
        nc.sync.dma_start(blb_sb[:], blb[:])
        brb_sb = cp.tile([P, F], bf, tag="brb")
        nc.sync.dma_start(brb_sb[:], brb[:])
        attb_sb = cp.tile([P, F], bf, tag="attb")
        nc.sync.dma_start(attb_sb[:], attb[:])
        biasb_sb = cp.tile([P, F], bf, tag="biasb")
        nc.sync.dma_start(biasb_sb[:], biasb[:])
        idx_sb = cp.tile([P, TT * 8], mybir.dt.int16, tag="idx")
        nc.sync.dma_start(idx_sb[:], idx_d[:])
        ident_sb = cp.tile([P, P], bf, tag="ident")
        nc.sync.dma_start(ident_sb[:], ident_d[:])
        if heads_out:
            headw_sb = cp.tile([P, 2 * C], bf, tag="headw")
            nc.sync.dma_start(headw_sb[:], headw[:])
            headb_sb = cp.tile([P, 2], f32, tag="headb")
            nc.sync.dma_start(headb_sb[:], headb[:])

        xr_sb = cp.tile([P, NB * F], bf, tag="xr")
        nc.vector.memset(xr_sb[:], 0.0)

        # ---- dense phase: xl_full = x @ wl + bl (all nodes), xr (own) ----
        with tc.tile_pool(name="dpsum", bufs=2, space="PSUM") as dps, \
             tc.tile_pool(name="dwork", bufs=3) as dw:
            xTr = xT.rearrange("(c k) f -> k c f", k=P)
            full_tiles = nn // P
            xlr = xl_full[0:full_tiles * P, :].rearrange(
                "(t p) f -> p t f", p=P)
            groups = [(m0, min(m0 + DB, full_tiles))
                      for m0 in range(0, full_tiles, DB)]
            for (m0, m1) in groups:
                db = m1 - m0
                lt = dw.tile([P, KC, DB * P], bf, tag="lt")
                nc.sync.dma_start(lt[:, :, :db * P],
                                  xTr[:, :, m0 * P:m1 * P])
                ot = dw.tile([P, DB, F], bf, tag="ot")
                for t in range(db):
                    ps = dps.tile([P, F], f32, tag="ps")
                    for c in range(KC):
                        nc.tensor.matmul(
                            ps[:], lhsT=lt[:, c, t * P:(t + 1) * P],
                            rhs=wl_sb[:, c, :],
                            start=(c == 0), stop=(c == KC - 1))
                    nc.vector.tensor_tensor(out=ot[:, t, :], in0=ps[:],
                                            in1=blb_sb[:], op=ALU.add)
                nc.sync.dma_start(xlr[:, m0:m1, :], ot[:, :db, :])
            # ragged tail tile + zero pad row
            r = nn - full_tiles * P
            if r:
                lt = dw.tile([P, KC, P], bf, tag="lt")
                nc.sync.dma_start(lt[:, :, :r],
                                  xTr[:, :, full_tiles * P:nn])
                ps = dps.tile([P, F], f32, tag="ps")
                for c in range(KC):
                    nc.tensor.matmul(ps[:r], lhsT=lt[:, c, :r],
                                     rhs=wl_sb[:, c, :],
                                     start=(c == 0), stop=(c == KC - 1))
                ot = dw.tile([P, F], bf, tag="ot2")
                nc.vector.tensor_tensor(out=ot[:r], in0=ps[:r],
                                        in1=blb_sb[:r], op=ALU.add)
                nc.sync.dma_start(xl_full[full_tiles * P:nn, :], ot[:r])
            zt = dw.tile([1, F], bf, tag="zt")
            nc.vector.memset(zt[:], 0.0)
            nc.sync.dma_start(xl_full[nn:nn + 1, :], zt[:])

            xoTr = xoT.rearrange("(c k) f -> k c f", k=P)
            bgroups = [(b0, min(b0 + DB, NB)) for b0 in range(0, NB, DB)]
            for (b0, b1) in bgroups:
                w = blk_rows[b1 - 1] + (b1 - 1 - b0) * P
                lt = dw.tile([P, KC, DB * P], bf, tag="lt")
                nc.sync.dma_start(lt[:, :, :w], xoTr[:, :, b0 * P:b0 * P + w])
                for b in range(b0, b1):
                    r = blk_rows[b]
                    o = (b - b0) * P
                    ps = dps.tile([P, F], f32, tag="ps")
                    for c in range(KC):
                        nc.tensor.matmul(ps[:r], lhsT=lt[:, c, o:o + r],
                                         rhs=wr_sb[:, c, :],
                                         start=(c == 0), stop=(c == KC - 1))
                    nc.vector.tensor_tensor(out=xr_sb[:r, b * F:(b + 1) * F],
                                            in0=ps[:r], in1=brb_sb[:r],
                                            op=ALU.add)

        tc.strict_bb_all_engine_barrier()

        # ---- edge phase (2-tile software pipeline over the tile stream) ----
        stream = []  # (block, global tile, first, last)
        g = 0
        for b in range(NB):
            for t in range(Tb[b]):
                stream.append((b, g, t == 0, t == Tb[b] - 1))
                g += 1
        Tt = len(stream)
        chunks = [(c0, min(c0 + GCH, Tt)) for c0 in range(0, Tt, GCH)]
        chunk_of = {}
        for ci, (c0, c1) in enumerate(chunks):
            for gg in range(c0, c1):
                chunk_of[gg] = ci

        gsems = [nc.alloc_semaphore(f"gsem{q}") for q in range(NQ)]

        with tc.tile_pool(name="gat", bufs=3) as gp, \
             tc.tile_pool(name="ohc", bufs=3) as ohcp, \
             tc.tile_pool(name="elp", bufs=3) as elp, \
             tc.tile_pool(name="msp", bufs=3) as msp, \
             tc.tile_pool(name="scp", bufs=4) as scp, \
             tc.tile_pool(name="blk", bufs=2) as blk, \
             tc.tile_pool(name="ps_s", bufs=3, space="PSUM") as psp, \
             tc.tile_pool(name="ps_out", bufs=2, space="PSUM") as ps_out_p, \
             tc.tile_pool(name="ps_den", bufs=2, space="PSUM") as ps_den_p:

            xg_tiles = {}      # chunk idx -> (tile, c0)
            qseq = [0] * NQ    # completed-gather count per queue
            chunk_sem = {}     # chunk idx -> (queue, wait value)
            waited = set()     # (engine, chunk) wait already emitted
            ohc_tiles = {}     # chunk idx -> (oh tile, ohT tile, c0)
            el_t, exb_t, ps_s_t = {}, {}, {}
            pout_b, pden_b = {}, {}

            def emit_chunk(ci):
                c0, c1 = chunks[ci]
                n_t = c1 - c0
                q = ci % NQ
                xg = gp.tile([P, GCH, F], bf, tag="xg")
                if PREP_GATHER:
                    nc.gpsimd.dma_gather(
                        xg[:, :n_t, :], xl_full[:],
                        idx_sb[:, c0 * 8:c1 * 8],
                        num_idxs=n_t * P, num_idxs_reg=n_t * P, elem_size=F,
                        single_packet=False, queue_num=q,
                        prepare_only=True, sem=gsems[q])
                    nc.gpsimd.trigger_dma(count=None, queue_num=q)
                    qseq[q] += 1
                    chunk_sem[ci] = (q, 16 * qseq[q])
                else:
                    nc.gpsimd.dma_gather(
                        xg[:, :n_t, :], xl_full[:],
                        idx_sb[:, c0 * 8:c1 * 8],
                        num_idxs=n_t * P, num_idxs_reg=n_t * P, elem_size=F,
                        single_packet=False, queue_num=q)
                xg_tiles[ci] = (xg, c0)
                ohc = ohcp.tile([P, GCH * P], bf, tag="ohc")
                nc.sync.dma_start(ohc[:, :n_t * P], oh_d[:, c0 * P:c1 * P])
                ohTc = ohcp.tile([P, GCH * P], bf, tag="ohTc")
                nc.sync.dma_start(ohTc[:, :n_t * P], ohT_d[:, c0 * P:c1 * P])
                ohc_tiles[ci] = (ohc, ohTc, c0)

            def xg_slice(gg):
                xg, c0 = xg_tiles[chunk_of[gg]]
                return xg[:, gg - c0, :]

            def oh_slices(gg):
                ohc, ohTc, c0 = ohc_tiles[chunk_of[gg]]
                o = (gg - c0) * P
                return ohc[:, o:o + P], ohTc[:, o:o + P]

            def block_end(b):
                r = blk_rows[b]
                den = blk.tile([P, H], f32, tag="den")
                nc.vector.tensor_scalar(out=den[:], in0=pden_b[b][:],
                                        scalar1=1e-16, scalar2=None,
                                        op0=ALU.add)
                rec = blk.tile([P, H], f32, tag="rec")
                nc.vector.reciprocal(rec[:], den[:])
                hb = blk.tile([P, F], bf, tag="hb")
                for h in range(H):
                    nc.scalar.activation(hb[:, h * C:(h + 1) * C],
                                         pout_b[b][:, h * C:(h + 1) * C],
                                         AF.Copy, scale=rec[:, h:h + 1])
                hc = blk.tile([P, F], bf, tag="hc")
                nc.vector.tensor_tensor(out=hc[:], in0=hb[:], in1=biasb_sb[:],
                                        op=ALU.add)
                if elu:
                    # h+1 is stored; host folds the -1 into layer-2 biases.
                    mn = blk.tile([P, F], bf, tag="mn")
                    nc.vector.tensor_scalar_min(mn[:], hc[:], 0.0)
                    en = blk.tile([P, F], bf, tag="en")
                    nc.scalar.activation(en[:], mn[:], AF.Exp)
                    mx = blk.tile([P, F], bf, tag="mx")
                    nc.vector.tensor_scalar_max(mx[:], hc[:], 0.0)
                    ho = blk.tile([P, F], bf, tag="ho")
                    nc.vector.tensor_tensor(out=ho[:], in0=mx[:], in1=en[:],
                                            op=ALU.add)
                    nc.sync.dma_start(h_d[b * P:b * P + r, :], ho[:r])
                elif heads_out:
                    for j, outd in enumerate([an_d, rc_d]):
                        tmp = blk.tile([P, C], bf, tag=f"ht{j}")
                        nc.vector.tensor_tensor(
                            out=tmp[:], in0=hc[:],
                            in1=headw_sb[:, j * C:(j + 1) * C], op=ALU.mult)
                        red = blk.tile([P, 1], f32, tag=f"hr{j}")
                        nc.vector.tensor_reduce(out=red[:], in_=tmp[:],
                                                axis=AX.X, op=ALU.add)
                        sg = blk.tile([P, 1], f32, tag=f"hs{j}")
                        nc.scalar.activation(sg[:], red[:], AF.Sigmoid,
                                             bias=headb_sb[:, j:j + 1])
                        nc.sync.dma_start(outd[b * P:b * P + r, :], sg[:r])
                else:
                    nc.sync.dma_start(h_d[b * P:b * P + r, :], hc[:r])
                del pout_b[b], pden_b[b]

            emit_chunk(0)
            if len(chunks) > 1:
                emit_chunk(1)

            for i in range(Tt + 2):
                if i < Tt:
                    b, gg, first, last = stream[i]
                    ci = chunk_of[gg]
                    if gg == chunks[ci][0] and ci >= 1 and ci + 1 < len(chunks):
                        emit_chunk(ci + 1)
                    # S: s = xr[dst] + xl[src] accumulated in PSUM
                    if PREP_GATHER and ("pe", ci) not in waited:
                        qn, val = chunk_sem[ci]
                        nc.tensor.wait_ge(gsems[qn], val)
                        waited.add(("pe", ci))
                    _, ohT_s = oh_slices(gg)
                    ps_s = psp.tile([P, F], f32, tag="s")
                    nc.tensor.matmul(ps_s[:], lhsT=ohT_s,
                                     rhs=xr_sb[:, b * F:(b + 1) * F],
                                     start=True, stop=False)
                    nc.tensor.matmul(ps_s[:], lhsT=ident_sb[:],
                                     rhs=xg_slice(gg), start=False, stop=True)
                    ps_s_t[gg] = ps_s

                j = i - 1
                if 0 <= j < Tt:
                    gj = stream[j][1]
                    # D2: att dot -> per-head scores
                    pr = msp.tile([P, F], bf, tag="pr")
                    nc.vector.tensor_tensor(out=pr[:], in0=el_t[gj],
                                            in1=attb_sb[:], op=ALU.mult)
                    sc = scp.tile([P, H], bf, tag="sc")
                    with nc.allow_low_precision(reason="bf16 scores validated"):
                        nc.vector.tensor_reduce(
                            out=sc[:],
                            in_=pr[:].rearrange("p (h c) -> p h c", c=C),
                            axis=AX.X, op=ALU.add)
                    # SC2: exp (f32 for DVE scalar muls, bf16 for matmul)
                    exf = scp.tile([P, H], f32, tag="exf")
                    nc.scalar.activation(exf[:], sc[:], AF.Exp)
                    exb = scp.tile([P, H], bf, tag="exb")
                    nc.scalar.copy(exb[:], exf[:])
                    exb_t[gj] = (exf, exb)

                if i < Tt:
                    # SC1: leaky relu (PSUM -> bf16)
                    gi = stream[i][1]
                    el = elp.tile([P, F], bf, tag="el")
                    nc.scalar.activation(el[:], ps_s_t[gi][:], AF.Prelu,
                                         alpha=NEG_SLOPE)
                    el_t[gi] = el
                    del ps_s_t[gi]

                k = i - 2
                if 0 <= k:
                    bk, gk, first, last = stream[k]
                    # D3: per-head message scaling
                    ck = chunk_of[gk]
                    if PREP_GATHER and ("dve", ck) not in waited:
                        qn, val = chunk_sem[ck]
                        nc.vector.wait_ge(gsems[qn], val)
                        waited.add(("dve", ck))
                    msg = msp.tile([P, F], bf, tag="msg")
                    exf, exb = exb_t[gk]
                    for h in range(H):
                        nc.vector.tensor_scalar_mul(
                            msg[:, h * C:(h + 1) * C],
                            xg_slice(gk)[:, h * C:(h + 1) * C],
                            exf[:, h:h + 1])
                    # B: segment sums
                    oh_s, _ = oh_slices(gk)
                    if first:
                        pout_b[bk] = ps_out_p.tile([P, F], f32, tag="pout",
                                                   name=f"pout{bk}")
                        pden_b[bk] = ps_den_p.tile([P, H], f32, tag="pden",
                                                   name=f"pden{bk}")
                    nc.tensor.matmul(pden_b[bk][:], lhsT=oh_s, rhs=exb[:],
                                     start=first, stop=last,
                                     skip_group_check=True)
                    nc.tensor.matmul(pout_b[bk][:], lhsT=oh_s, rhs=msg[:],
                                     start=first, stop=last,
                                     skip_group_check=True)
                    del exb_t[gk]
                    el_t.pop(gk, None)
                    if last:
                        block_end(bk)

    nc.compile()
    return nc


# ---------------------------------------------------------------------------
# Host orchestration
# ---------------------------------------------------------------------------

def _rep(v):
    """Replicate a 1-D param vector across 128 partitions (bf16)."""
    v = np.asarray(v, dtype=np.float32).reshape(-1)
    return np.tile(v[None, :], (P, 1)).astype(BF16)


TRACE = False          # set by test harness to capture NTFF profiles
LAST_RESULTS = []      # BassKernelResults of the last kernel() call


def run_spmd(nc, in_maps, trace=False, trace_kwargs=None):
    from concourse import bass_utils
    res = bass_utils.run_bass_kernel_spmd(
        nc, in_maps, core_ids=list(range(len(in_maps))), trace=trace or TRACE,
        **(trace_kwargs or {}))
    LAST_RESULTS.append(res)
    return res


def _bf(a):
    return np.ascontiguousarray(np.asarray(a).astype(BF16))


def kernel(x, edge_index, W1l, b1l, W1r, b1r, att1, bias1,
           W2l, b2l, W2r, b2r, att2, bias2, Wa, ba, Wrc, brc):
    x = np.asarray(x, dtype=np.float32)
    Tb, per_core = prep_edges(edge_index)

    ident = np.eye(P, dtype=np.float32).astype(BF16)
    consts = {"ident": ident}

    nc1 = _get_program(1, tuple(Tb))
    xb = x.astype(BF16)
    common1 = {
        "xT": np.ascontiguousarray(xb.T),
        "wl": _bf(W1l), "wr": _bf(W1r),
        "blb": _rep(b1l), "brb": _rep(b1r),
        "attb": _rep(np.asarray(att1, np.float32).reshape(-1)),
        "biasb": _rep(bias1),
        **consts,
    }
    in_maps = []
    for c in range(NCORES):
        m = dict(common1)
        m["xoT"] = np.ascontiguousarray(xb[c * NPC:(c + 1) * NPC].T)
        m["idx"] = per_core[c]["idx"]
        m["oh"] = per_core[c]["oh"]
        m["ohT"] = per_core[c]["ohT"]
        in_maps.append(m)
    LAST_RESULTS.clear()
    res1 = run_spmd(nc1, in_maps)
    h = np.concatenate([np.asarray(res1.results[c]["h_own"])
                        for c in range(NCORES)], axis=0)  # = elu(h_true)+1

    nc2 = _get_program(2, tuple(Tb))
    # h_true = h - 1: fold the -1 into the layer-2 linear biases.
    b2l_eff = np.asarray(b2l, np.float32) - np.asarray(W2l, np.float32).sum(0)
    b2r_eff = np.asarray(b2r, np.float32) - np.asarray(W2r, np.float32).sum(0)
    common2 = {
        "xT": np.ascontiguousarray(h.T),
        "wl": _bf(W2l), "wr": _bf(W2r),
        "blb": _rep(b2l_eff), "brb": _rep(b2r_eff),
        "attb": _rep(np.asarray(att2, np.float32).reshape(-1)),
        "biasb": _rep(bias2),
        "headw": np.concatenate(
            [_rep(np.asarray(Wa, np.float32).reshape(-1)),
             _rep(np.asarray(Wrc, np.float32).reshape(-1))], axis=1),
        "headb": np.concatenate(
            [np.full((P, 1), np.float32(np.asarray(ba).reshape(())), np.float32),
             np.full((P, 1), np.float32(np.asarray(brc).reshape(())), np.float32)],
            axis=1),
        **consts,
    }
    in_maps2 = []
    for c in range(NCORES):
        m = dict(common2)
        m["xoT"] = np.ascontiguousarray(h[c * NPC:(c + 1) * NPC].T)
        m["idx"] = per_core[c]["idx"]
        m["oh"] = per_core[c]["oh"]
        m["ohT"] = per_core[c]["ohT"]
        in_maps2.append(m)
    res2 = run_spmd(nc2, in_maps2)
    an = np.concatenate([np.asarray(res2.results[c]["an"])
                         for c in range(NCORES)], axis=0)
    rc = np.concatenate([np.asarray(res2.results[c]["rc"])
                         for c in range(NCORES)], axis=0)
    return an, rc


_PROGRAMS = {}


def _get_program(layer, tb_key):
    key = (layer, tb_key)
    if key not in _PROGRAMS:
        if layer == 1:
            _PROGRAMS[key] = build_layer(N_NODES, NPC, IN_DIM, HEADS,
                                         list(tb_key), elu=True, heads_out=False)
        else:
            _PROGRAMS[key] = build_layer(N_NODES, NPC, HEADS * HID, 1,
                                         list(tb_key), elu=False, heads_out=True)
    return _PROGRAMS[key]


# revision 12
# speedup vs baseline: 1.9939x; 1.9939x over previous
"""Trainium2 Bass kernel for a 2-layer GATv2 (DependencyGraphAnalyzer).

Strategy (8 cores, SPMD, bf16):
  - Host sorts edges by dst and shards them by dst-node range: core c owns
    nodes [c*2500, (c+1)*2500) and every edge pointing into that range.
    Softmax segments (per-dst) are therefore entirely core-local.
  - Everything feature-sized runs in bf16 (validated ~1.3e-3 end-to-end
    rel err vs the 2e-2 gate): 4x faster matmuls, faster DVE, half the
    gather bytes.
  - Each core computes the full source transform xl = x@Wl+bl into an HBM
    bf16 table and its own range's xr = x@Wr+br into SBUF. Dense-phase
    DMAs are batched 4 node-tiles per transfer (sync-engine dispatch is
    ~700ns per dma_start).
  - The per-tile one-hot scatter matrices (oh and its transpose) depend
    only on the static dst pattern: the host precomputes them as bf16
    tables, streamed per 12-tile chunk (saves 2 DVE builds per tile).
  - Edge phase runs per 128-edge tile with a 2-tile software pipeline:
    dma_gather of xl[src] rows in PREPARE_ONLY mode (desc-gen only on
    gpsimd, transfers fly on 4 rotating SWDGE queues), one-hot scatter
    matmuls accumulated in PSUM (s = xr[dst] + xl[src] via ohT- and
    identity-matmuls), Prelu on the scalar engine, att-dot on DVE, exp on
    scalar, per-head msg scaling on DVE, and segment-sum (denominator +
    message) matmuls into PSUM.
  - Softmax denominator applied after aggregation (constant per segment);
    max-subtraction skipped (scores are O(1); exp is safe in fp32 range).
  - ELU's "-1" is folded into the layer-2 biases host-side.
  - Two launches; host all-gathers the hidden state h between the layers.
"""

import numpy as np
import ml_dtypes

BF16 = ml_dtypes.bfloat16

# Problem constants (hardcoded; kernel.py must be self-contained).
N_NODES = 20000
N_EDGES = 320000
IN_DIM = 256
HID = 128
HEADS = 4
NEG_SLOPE = 0.2
NCORES = 8
NPC = N_NODES // NCORES  # 2500 own nodes per core
P = 128
GCH = 12                 # gather chunk: tiles per dma_gather call
NQ = 4                   # SWDGE queues for gathers
DB = 4                   # dense phase: node tiles per batched DMA
PREP_GATHER = False       # prepare_only + trigger_dma gather pipelining


# ---------------------------------------------------------------------------
# Host-side edge preprocessing
# ---------------------------------------------------------------------------

def prep_edges(edge_index, n_nodes=N_NODES, ncores=NCORES):
    """Sort edges by dst, shard by dst range, pad each (core, block) segment
    to a common per-block tile count, and build the device index arrays.

    Returns (Tb, per_core) where Tb[b] is the number of 128-edge tiles of
    block b (shared by all cores) and per_core[c] is a dict with:
      idx : [128, TT*8] int16  wrapped dma_gather indices (pad -> n_nodes)
      oh  : [128, TT*128] bf16 one-hot; oh[e, g*128+j] = dst(g,e) == j
      ohT : [128, TT*128] bf16 transposed; ohT[j, g*128+e] = dst(g,e) == j
    """
    npc = n_nodes // ncores
    nb = (npc + P - 1) // P
    src = np.asarray(edge_index[0], dtype=np.int64)
    dst = np.asarray(edge_index[1], dtype=np.int64)
    order = np.argsort(dst, kind="stable")
    src_s, dst_s = src[order], dst[order]

    core_of = dst_s // npc
    blk_of = core_of * nb + (dst_s - core_of * npc) // P
    counts = np.bincount(blk_of, minlength=ncores * nb).reshape(ncores, nb)
    ends = np.cumsum(counts.reshape(-1)).reshape(ncores, nb)
    starts = ends - counts

    tiles = (counts + P - 1) // P            # [ncores, nb]
    Tb = np.maximum(tiles.max(axis=0), 1)    # shared per-block tile count
    TT = int(Tb.sum())
    offs = np.concatenate([[0], np.cumsum(Tb)[:-1]])  # tile offset per block

    jj = np.arange(P, dtype=np.int32)
    per_core = []
    for c in range(ncores):
        idx_flat = np.full(TT * P, n_nodes, dtype=np.int64)  # pad -> zero row
        dloc_flat = np.full(TT * P, -1, dtype=np.int32)      # pad -> -1
        for b in range(nb):
            s, e = starts[c, b], ends[c, b]
            o = offs[b] * P
            idx_flat[o:o + e - s] = src_s[s:e]
            dloc_flat[o:o + e - s] = (dst_s[s:e] - c * npc - b * P)
        # Wrap gather indices: within a gather call over a 128-aligned
        # sub-range, index j lives at [j % 16, j // 16], replicated to 128
        # partitions. The wrap is position-linear, so any 128-aligned
        # sub-range of the stream can be sliced out for one call.
        idx_w = np.tile(idx_flat.astype(np.int16).reshape(TT * 8, 16).T, (8, 1))
        dloc = dloc_flat.reshape(TT, P)      # [tile, edge lane]
        oh = (dloc[:, :, None] == jj[None, None, :])            # [g, e, j]
        oh_h = np.ascontiguousarray(
            oh.transpose(1, 0, 2).reshape(P, TT * P)).astype(BF16)
        ohT_h = np.ascontiguousarray(
            oh.transpose(2, 0, 1).reshape(P, TT * P)).astype(BF16)
        per_core.append({"idx": np.ascontiguousarray(idx_w),
                         "oh": oh_h, "ohT": ohT_h})
    return [int(t) for t in Tb], per_core


# ---------------------------------------------------------------------------
# Bass program builder (one GATv2 layer, optionally + heads)
# ---------------------------------------------------------------------------

def build_layer(nn, npc, in_dim, heads, Tb, elu, heads_out, num_devices=NCORES):
    import concourse.bacc as bacc
    import concourse.tile as tile
    import concourse.mybir as mybir
    from contextlib import ExitStack

    f32 = mybir.dt.float32
    bf = mybir.dt.bfloat16
    AF = mybir.ActivationFunctionType
    ALU = mybir.AluOpType
    AX = mybir.AxisListType

    C = 128
    H = heads
    F = H * C
    KC = in_dim // P
    NB = len(Tb)
    TT = sum(Tb)
    blk_rows = [min(P, npc - b * P) for b in range(NB)]
    n_node_tiles = (nn + P - 1) // P

    nc = bacc.Bacc("TRN2", target_bir_lowering=False, debug=False,
                   num_devices=num_devices, num_swdge_queues=NQ)

    # --- DRAM tensors ---
    xT = nc.dram_tensor("xT", [in_dim, nn], bf, kind="ExternalInput").ap()
    xoT = nc.dram_tensor("xoT", [in_dim, npc], bf, kind="ExternalInput").ap()
    wl = nc.dram_tensor("wl", [in_dim, F], bf, kind="ExternalInput").ap()
    wr = nc.dram_tensor("wr", [in_dim, F], bf, kind="ExternalInput").ap()
    blb = nc.dram_tensor("blb", [P, F], bf, kind="ExternalInput").ap()
    brb = nc.dram_tensor("brb", [P, F], bf, kind="ExternalInput").ap()
    attb = nc.dram_tensor("attb", [P, F], bf, kind="ExternalInput").ap()
    biasb = nc.dram_tensor("biasb", [P, F], bf, kind="ExternalInput").ap()
    idx_d = nc.dram_tensor("idx", [P, TT * 8], mybir.dt.int16,
                           kind="ExternalInput").ap()
    oh_d = nc.dram_tensor("oh", [P, TT * P], bf, kind="ExternalInput").ap()
    ohT_d = nc.dram_tensor("ohT", [P, TT * P], bf, kind="ExternalInput").ap()
    ident_d = nc.dram_tensor("ident", [P, P], bf, kind="ExternalInput").ap()
    if heads_out:
        headw = nc.dram_tensor("headw", [P, 2 * C], bf, kind="ExternalInput").ap()
        headb = nc.dram_tensor("headb", [P, 2], f32, kind="ExternalInput").ap()
        an_d = nc.dram_tensor("an", [npc, 1], f32, kind="ExternalOutput").ap()
        rc_d = nc.dram_tensor("rc", [npc, 1], f32, kind="ExternalOutput").ap()
    else:
        h_d = nc.dram_tensor("h_own", [npc, F], bf, kind="ExternalOutput").ap()
    xl_full = nc.dram_tensor("xl_full", [nn + 1, F], bf, kind="Internal").ap()

    with tile.TileContext(nc) as tc, ExitStack() as es:
        cp = es.enter_context(tc.tile_pool(name="const", bufs=1))

        wl_sb = cp.tile([P, KC, F], bf, tag="wl")
        nc.sync.dma_start(wl_sb[:], wl.rearrange("(c k) f -> k c f", k=P))
        wr_sb = cp.tile([P, KC, F], bf, tag="wr")
        nc.sync.dma_start(wr_sb[:], wr.rearrange("(c k) f -> k c f", k=P))
        blb_sb = cp.tile([P, F], bf, tag="blb")
        nc.sync.dma_start(blb_sb[:], blb[:])
        brb_sb = cp.tile([P, F], bf, tag="brb")
        nc.sync.dma_start(brb_sb[:], brb[:])
        attb_sb = cp.tile([P, F], bf, tag="attb")
        nc.sync.dma_start(attb_sb[:], attb[:])
        biasb_sb = cp.tile([P, F], bf, tag="biasb")
        nc.sync.dma_start(biasb_sb[:], biasb[:])
        idx_sb = cp.tile([P, TT * 8], mybir.dt.int16, tag="idx")
        nc.sync.dma_start(idx_sb[:], idx_d[:])
        ident_sb = cp.tile([P, P], bf, tag="ident")
        nc.sync.dma_start(ident_sb[:], ident_d[:])
        if heads_out:
            headw_sb = cp.tile([P, 2 * C], bf, tag="headw")
            nc.sync.dma_start(headw_sb[:], headw[:])
            headb_sb = cp.tile([P, 2], f32, tag="headb")
            nc.sync.dma_start(headb_sb[:], headb[:])

        xr_sb = cp.tile([P, NB * F], bf, tag="xr")
        nc.vector.memset(xr_sb[:], 0.0)

        # ---- dense phase: xl_full = x @ wl + bl (all nodes), xr (own) ----
        with tc.tile_pool(name="dpsum", bufs=2, space="PSUM") as dps, \
             tc.tile_pool(name="dwork", bufs=3) as dw:
            xTr = xT.rearrange("(c k) f -> k c f", k=P)
            full_tiles = nn // P
            xlr = xl_full[0:full_tiles * P, :].rearrange(
                "(t p) f -> p t f", p=P)
            groups = [(m0, min(m0 + DB, full_tiles))
                      for m0 in range(0, full_tiles, DB)]
            for (m0, m1) in groups:
                db = m1 - m0
                lt = dw.tile([P, KC, DB * P], bf, tag="lt")
                nc.sync.dma_start(lt[:, :, :db * P],
                                  xTr[:, :, m0 * P:m1 * P])
                ot = dw.tile([P, DB, F], bf, tag="ot")
                for t in range(db):
                    ps = dps.tile([P, F], f32, tag="ps")
                    for c in range(KC):
                        nc.tensor.matmul(
                            ps[:], lhsT=lt[:, c, t * P:(t + 1) * P],
                            rhs=wl_sb[:, c, :],
                            start=(c == 0), stop=(c == KC - 1))
                    nc.vector.tensor_tensor(out=ot[:, t, :], in0=ps[:],
                                            in1=blb_sb[:], op=ALU.add)
                nc.sync.dma_start(xlr[:, m0:m1, :], ot[:, :db, :])
            # ragged tail tile + zero pad row
            r = nn - full_tiles * P
            if r:
                lt = dw.tile([P, KC, P], bf, tag="lt")
                nc.sync.dma_start(lt[:, :, :r],
                                  xTr[:, :, full_tiles * P:nn])
                ps = dps.tile([P, F], f32, tag="ps")
                for c in range(KC):
                    nc.tensor.matmul(ps[:r], lhsT=lt[:, c, :r],
                                     rhs=wl_sb[:, c, :],
                                     start=(c == 0), stop=(c == KC - 1))
                ot = dw.tile([P, F], bf, tag="ot2")
                nc.vector.tensor_tensor(out=ot[:r], in0=ps[:r],
                                        in1=blb_sb[:r], op=ALU.add)
                nc.sync.dma_start(xl_full[full_tiles * P:nn, :], ot[:r])
            zt = dw.tile([1, F], bf, tag="zt")
            nc.vector.memset(zt[:], 0.0)
            nc.sync.dma_start(xl_full[nn:nn + 1, :], zt[:])

            xoTr = xoT.rearrange("(c k) f -> k c f", k=P)
            bgroups = [(b0, min(b0 + DB, NB)) for b0 in range(0, NB, DB)]
            for (b0, b1) in bgroups:
                w = blk_rows[b1 - 1] + (b1 - 1 - b0) * P
                lt = dw.tile([P, KC, DB * P], bf, tag="lt")
                nc.sync.dma_start(lt[:, :, :w], xoTr[:, :, b0 * P:b0 * P + w])
                for b in range(b0, b1):
                    r = blk_rows[b]
                    o = (b - b0) * P
                    ps = dps.tile([P, F], f32, tag="ps")
                    for c in range(KC):
                        nc.tensor.matmul(ps[:r], lhsT=lt[:, c, o:o + r],
                                         rhs=wr_sb[:, c, :],
                                         start=(c == 0), stop=(c == KC - 1))
                    nc.vector.tensor_tensor(out=xr_sb[:r, b * F:(b + 1) * F],
                                            in0=ps[:r], in1=brb_sb[:r],
                                            op=ALU.add)

        tc.strict_bb_all_engine_barrier()

        # ---- edge phase (2-tile software pipeline over the tile stream) ----
        stream = []  # (block, global tile, first, last)
        g = 0
        for b in range(NB):
            for t in range(Tb[b]):
                stream.append((b, g, t == 0, t == Tb[b] - 1))
                g += 1
        Tt = len(stream)
        chunks = [(c0, min(c0 + GCH, Tt)) for c0 in range(0, Tt, GCH)]
        chunk_of = {}
        for ci, (c0, c1) in enumerate(chunks):
            for gg in range(c0, c1):
                chunk_of[gg] = ci

        gsems = [nc.alloc_semaphore(f"gsem{q}") for q in range(NQ)]

        with tc.tile_pool(name="gat", bufs=3) as gp, \
             tc.tile_pool(name="ohc", bufs=3) as ohcp, \
             tc.tile_pool(name="elp", bufs=3) as elp, \
             tc.tile_pool(name="msp", bufs=3) as msp, \
             tc.tile_pool(name="scp", bufs=4) as scp, \
             tc.tile_pool(name="blk", bufs=2) as blk, \
             tc.tile_pool(name="ps_s", bufs=3, space="PSUM") as psp, \
             tc.tile_pool(name="ps_out", bufs=2, space="PSUM") as ps_out_p, \
             tc.tile_pool(name="ps_den", bufs=2, space="PSUM") as ps_den_p:

            xg_tiles = {}      # chunk idx -> (tile, c0)
            qseq = [0] * NQ    # completed-gather count per queue
            chunk_sem = {}     # chunk idx -> (queue, wait value)
            waited = set()     # (engine, chunk) wait already emitted
            ohc_tiles = {}     # chunk idx -> (oh tile, ohT tile, c0)
            el_t, exb_t, ps_s_t = {}, {}, {}
            pout_b, pden_b = {}, {}

            def emit_chunk(ci):
                c0, c1 = chunks[ci]
                n_t = c1 - c0
                q = ci % NQ
                xg = gp.tile([P, GCH, F], bf, tag="xg")
                if PREP_GATHER:
                    nc.gpsimd.dma_gather(
                        xg[:, :n_t, :], xl_full[:],
                        idx_sb[:, c0 * 8:c1 * 8],
                        num_idxs=n_t * P, num_idxs_reg=n_t * P, elem_size=F,
                        single_packet=False, queue_num=q,
                        prepare_only=True, sem=gsems[q])
                    nc.gpsimd.trigger_dma(count=None, queue_num=q)
                    qseq[q] += 1
                    chunk_sem[ci] = (q, 16 * qseq[q])
                else:
                    nc.gpsimd.dma_gather(
                        xg[:, :n_t, :], xl_full[:],
                        idx_sb[:, c0 * 8:c1 * 8],
                        num_idxs=n_t * P, num_idxs_reg=n_t * P, elem_size=F,
                        single_packet=False, queue_num=q)
                xg_tiles[ci] = (xg, c0)
                ohc = ohcp.tile([P, GCH * P], bf, tag="ohc")
                nc.sync.dma_start(ohc[:, :n_t * P], oh_d[:, c0 * P:c1 * P])
                ohTc = ohcp.tile([P, GCH * P], bf, tag="ohTc")
                nc.sync.dma_start(ohTc[:, :n_t * P], ohT_d[:, c0 * P:c1 * P])
                ohc_tiles[ci] = (ohc, ohTc, c0)

            def xg_slice(gg):
                xg, c0 = xg_tiles[chunk_of[gg]]
                return xg[:, gg - c0, :]

            def oh_slices(gg):
                ohc, ohTc, c0 = ohc_tiles[chunk_of[gg]]
                o = (gg - c0) * P
                return ohc[:, o:o + P], ohTc[:, o:o + P]

            def block_end(b):
                r = blk_rows[b]
                den = blk.tile([P, H], f32, tag="den")
                nc.vector.tensor_scalar(out=den[:], in0=pden_b[b][:],
                                        scalar1=1e-16, scalar2=None,
                                        op0=ALU.add)
                rec = blk.tile([P, H], f32, tag="rec")
                nc.vector.reciprocal(rec[:], den[:])
                hb = blk.tile([P, F], bf, tag="hb")
                for h in range(H):
                    nc.scalar.activation(hb[:, h * C:(h + 1) * C],
                                         pout_b[b][:, h * C:(h + 1) * C],
                                         AF.Copy, scale=rec[:, h:h + 1])
                hc = blk.tile([P, F], bf, tag="hc")
                nc.vector.tensor_tensor(out=hc[:], in0=hb[:], in1=biasb_sb[:],
                                        op=ALU.add)
                if elu:
                    # h+1 is stored; host folds the -1 into layer-2 biases.
                    mn = blk.tile([P, F], bf, tag="mn")
                    nc.vector.tensor_scalar_min(mn[:], hc[:], 0.0)
                    en = blk.tile([P, F], bf, tag="en")
                    nc.scalar.activation(en[:], mn[:], AF.Exp)
                    mx = blk.tile([P, F], bf, tag="mx")
                    nc.vector.tensor_scalar_max(mx[:], hc[:], 0.0)
                    ho = blk.tile([P, F], bf, tag="ho")
                    nc.vector.tensor_tensor(out=ho[:], in0=mx[:], in1=en[:],
                                            op=ALU.add)
                    nc.sync.dma_start(h_d[b * P:b * P + r, :], ho[:r])
                elif heads_out:
                    for j, outd in enumerate([an_d, rc_d]):
                        tmp = blk.tile([P, C], bf, tag=f"ht{j}")
                        nc.vector.tensor_tensor(
                            out=tmp[:], in0=hc[:],
                            in1=headw_sb[:, j * C:(j + 1) * C], op=ALU.mult)
                        red = blk.tile([P, 1], f32, tag=f"hr{j}")
                        nc.vector.tensor_reduce(out=red[:], in_=tmp[:],
                                                axis=AX.X, op=ALU.add)
                        sg = blk.tile([P, 1], f32, tag=f"hs{j}")
                        nc.scalar.activation(sg[:], red[:], AF.Sigmoid,
                                             bias=headb_sb[:, j:j + 1])
                        nc.sync.dma_start(outd[b * P:b * P + r, :], sg[:r])
                else:
                    nc.sync.dma_start(h_d[b * P:b * P + r, :], hc[:r])
                del pout_b[b], pden_b[b]

            emit_chunk(0)
            if len(chunks) > 1:
                emit_chunk(1)

            for i in range(Tt + 2):
                if i < Tt:
                    b, gg, first, last = stream[i]
                    ci = chunk_of[gg]
                    if gg == chunks[ci][0] and ci >= 1 and ci + 1 < len(chunks):
                        emit_chunk(ci + 1)
                    # S: s = xr[dst] + xl[src] accumulated in PSUM
                    if PREP_GATHER and ("pe", ci) not in waited:
                        qn, val = chunk_sem[ci]
                        nc.tensor.wait_ge(gsems[qn], val)
                        waited.add(("pe", ci))
                    _, ohT_s = oh_slices(gg)
                    ps_s = psp.tile([P, F], f32, tag="s")
                    nc.tensor.matmul(ps_s[:], lhsT=ohT_s,
                                     rhs=xr_sb[:, b * F:(b + 1) * F],
                                     start=True, stop=False)
                    nc.tensor.matmul(ps_s[:], lhsT=ident_sb[:],
                                     rhs=xg_slice(gg), start=False, stop=True)
                    ps_s_t[gg] = ps_s

                j = i - 1
                if 0 <= j < Tt:
                    gj = stream[j][1]
                    # D2: att dot -> per-head scores
                    pr = msp.tile([P, F], bf, tag="pr")
                    nc.vector.tensor_tensor(out=pr[:], in0=el_t[gj],
                                            in1=attb_sb[:], op=ALU.mult)
                    sc = scp.tile([P, H], bf, tag="sc")
                    with nc.allow_low_precision(reason="bf16 scores validated"):
                        nc.vector.tensor_reduce(
                            out=sc[:],
                            in_=pr[:].rearrange("p (h c) -> p h c", c=C),
                            axis=AX.X, op=ALU.add)
                    # SC2: exp (f32 for DVE scalar muls, bf16 for matmul)
                    exf = scp.tile([P, H], f32, tag="exf")
                    nc.scalar.activation(exf[:], sc[:], AF.Exp)
                    exb = scp.tile([P, H], bf, tag="exb")
                    nc.scalar.copy(exb[:], exf[:])
                    exb_t[gj] = (exf, exb)

                if i < Tt:
                    # SC1: leaky relu (PSUM -> bf16)
                    gi = stream[i][1]
                    el = elp.tile([P, F], bf, tag="el")
                    nc.scalar.activation(el[:], ps_s_t[gi][:], AF.Prelu,
                                         alpha=NEG_SLOPE)
                    el_t[gi] = el
                    del ps_s_t[gi]

                k = i - 2
                if 0 <= k:
                    bk, gk, first, last = stream[k]
                    # D3: per-head message scaling
                    ck = chunk_of[gk]
                    if PREP_GATHER and ("dve", ck) not in waited:
                        qn, val = chunk_sem[ck]
                        nc.vector.wait_ge(gsems[qn], val)
                        waited.add(("dve", ck))
                    msg = msp.tile([P, F], bf, tag="msg")
                    exf, exb = exb_t[gk]
                    for h in range(H):
                        nc.vector.tensor_scalar_mul(
                            msg[:, h * C:(h + 1) * C],
                            xg_slice(gk)[:, h * C:(h + 1) * C],
                            exf[:, h:h + 1])
                    # B: segment sums
                    oh_s, _ = oh_slices(gk)
                    if first:
                        pout_b[bk] = ps_out_p.tile([P, F], f32, tag="pout",
                                                   name=f"pout{bk}")
                        pden_b[bk] = ps_den_p.tile([P, H], f32, tag="pden",
                                                   name=f"pden{bk}")
                    nc.tensor.matmul(pden_b[bk][:], lhsT=oh_s, rhs=exb[:],
                                     start=first, stop=last,
                                     skip_group_check=True)
                    nc.tensor.matmul(pout_b[bk][:], lhsT=oh_s, rhs=msg[:],
                                     start=first, stop=last,
                                     skip_group_check=True)
                    del exb_t[gk]
                    el_t.pop(gk, None)
                    if last:
                        block_end(bk)

    nc.compile()
    return nc


# ---------------------------------------------------------------------------
# Host orchestration
# ---------------------------------------------------------------------------

def _rep(v):
    """Replicate a 1-D param vector across 128 partitions (bf16)."""
    v = np.asarray(v, dtype=np.float32).reshape(-1)
    return np.tile(v[None, :], (P, 1)).astype(BF16)


TRACE = False          # set by test harness to capture NTFF profiles
TRACE_LAUNCHES = (0, 1)  # which launches to trace when TRACE is on
LAST_RESULTS = []      # BassKernelResults of the last kernel() call


def run_spmd(nc, in_maps, trace=False, trace_kwargs=None):
    from concourse import bass_utils
    launch = len(LAST_RESULTS)
    do_trace = (trace or TRACE) and launch in TRACE_LAUNCHES
    res = bass_utils.run_bass_kernel_spmd(
        nc, in_maps, core_ids=list(range(len(in_maps))), trace=do_trace,
        **(trace_kwargs or {}))
    LAST_RESULTS.append(res)
    return res


def _bf(a):
    return np.ascontiguousarray(np.asarray(a).astype(BF16))


def kernel(x, edge_index, W1l, b1l, W1r, b1r, att1, bias1,
           W2l, b2l, W2r, b2r, att2, bias2, Wa, ba, Wrc, brc):
    x = np.asarray(x, dtype=np.float32)
    Tb, per_core = prep_edges(edge_index)

    ident = np.eye(P, dtype=np.float32).astype(BF16)
    consts = {"ident": ident}

    nc1 = _get_program(1, tuple(Tb))
    xb = x.astype(BF16)
    common1 = {
        "xT": np.ascontiguousarray(xb.T),
        "wl": _bf(W1l), "wr": _bf(W1r),
        "blb": _rep(b1l), "brb": _rep(b1r),
        "attb": _rep(np.asarray(att1, np.float32).reshape(-1)),
        "biasb": _rep(bias1),
        **consts,
    }
    in_maps = []
    for c in range(NCORES):
        m = dict(common1)
        m["xoT"] = np.ascontiguousarray(xb[c * NPC:(c + 1) * NPC].T)
        m["idx"] = per_core[c]["idx"]
        m["oh"] = per_core[c]["oh"]
        m["ohT"] = per_core[c]["ohT"]
        in_maps.append(m)
    LAST_RESULTS.clear()
    res1 = run_spmd(nc1, in_maps)
    h = np.concatenate([np.asarray(res1.results[c]["h_own"])
                        for c in range(NCORES)], axis=0)  # = elu(h_true)+1

    nc2 = _get_program(2, tuple(Tb))
    # h_true = h - 1: fold the -1 into the layer-2 linear biases.
    b2l_eff = np.asarray(b2l, np.float32) - np.asarray(W2l, np.float32).sum(0)
    b2r_eff = np.asarray(b2r, np.float32) - np.asarray(W2r, np.float32).sum(0)
    common2 = {
        "xT": np.ascontiguousarray(h.T),
        "wl": _bf(W2l), "wr": _bf(W2r),
        "blb": _rep(b2l_eff), "brb": _rep(b2r_eff),
        "attb": _rep(np.asarray(att2, np.float32).reshape(-1)),
        "biasb": _rep(bias2),
        "headw": np.concatenate(
            [_rep(np.asarray(Wa, np.float32).reshape(-1)),
             _rep(np.asarray(Wrc, np.float32).reshape(-1))], axis=1),
        "headb": np.concatenate(
            [np.full((P, 1), np.float32(np.asarray(ba).reshape(())), np.float32),
             np.full((P, 1), np.float32(np.asarray(brc).reshape(())), np.float32)],
            axis=1),
        **consts,
    }
    in_maps2 = []
    for c in range(NCORES):
        m = dict(common2)
        m["xoT"] = np.ascontiguousarray(h[c * NPC:(c + 1) * NPC].T)
        m["idx"] = per_core[c]["idx"]
        m["oh"] = per_core[c]["oh"]
        m["ohT"] = per_core[c]["ohT"]
        in_maps2.append(m)
    res2 = run_spmd(nc2, in_maps2)
    an = np.concatenate([np.asarray(res2.results[c]["an"])
                         for c in range(NCORES)], axis=0)
    rc = np.concatenate([np.asarray(res2.results[c]["rc"])
                         for c in range(NCORES)], axis=0)
    return an, rc


_PROGRAMS = {}


def _get_program(layer, tb_key):
    key = (layer, tb_key)
    if key not in _PROGRAMS:
        if layer == 1:
            _PROGRAMS[key] = build_layer(N_NODES, NPC, IN_DIM, HEADS,
                                         list(tb_key), elu=True, heads_out=False)
        else:
            _PROGRAMS[key] = build_layer(N_NODES, NPC, HEADS * HID, 1,
                                         list(tb_key), elu=False, heads_out=True)
    return _PROGRAMS[key]


# revision 13
# speedup vs baseline: 2.0964x; 1.0514x over previous
"""Trainium2 Bass kernel for a 2-layer GATv2 (DependencyGraphAnalyzer).

Strategy (8 cores, SPMD, bf16):
  - Host sorts edges by dst and shards them by dst-node range: core c owns
    nodes [c*2500, (c+1)*2500) and every edge pointing into that range.
    Softmax segments (per-dst) are therefore entirely core-local.
  - Everything feature-sized runs in bf16 (validated ~1.3e-3 end-to-end
    rel err vs the 2e-2 gate): 4x faster matmuls, faster DVE, half the
    gather bytes.
  - Each core computes the full source transform xl = x@Wl+bl into an HBM
    bf16 table and its own range's xr = x@Wr+br into SBUF. Dense-phase
    DMAs are batched 4 node-tiles per transfer (sync-engine dispatch is
    ~700ns per dma_start).
  - The per-tile one-hot scatter matrices (oh and its transpose) depend
    only on the static dst pattern: the host precomputes them as bf16
    tables, streamed per 12-tile chunk (saves 2 DVE builds per tile).
  - Edge phase runs per 128-edge tile with a 2-tile software pipeline:
    dma_gather of xl[src] rows in PREPARE_ONLY mode (desc-gen only on
    gpsimd, transfers fly on 4 rotating SWDGE queues), one-hot scatter
    matmuls accumulated in PSUM (s = xr[dst] + xl[src] via ohT- and
    identity-matmuls), Prelu on the scalar engine, att-dot on DVE, exp on
    scalar, per-head msg scaling on DVE, and segment-sum (denominator +
    message) matmuls into PSUM.
  - Softmax denominator applied after aggregation (constant per segment);
    max-subtraction skipped (scores are O(1); exp is safe in fp32 range).
  - ELU's "-1" is folded into the layer-2 biases host-side.
  - Two launches; host all-gathers the hidden state h between the layers.
"""

import numpy as np
import ml_dtypes

BF16 = ml_dtypes.bfloat16

# Problem constants (hardcoded; kernel.py must be self-contained).
N_NODES = 20000
N_EDGES = 320000
IN_DIM = 256
HID = 128
HEADS = 4
NEG_SLOPE = 0.2
NCORES = 8
NPC = N_NODES // NCORES  # 2500 own nodes per core
P = 128
GCH = 12                 # gather chunk: tiles per dma_gather call
NQ = 4                   # SWDGE queues for gathers
DB = 4                   # dense phase: node tiles per batched DMA
PREP_GATHER = False       # prepare_only + trigger_dma gather pipelining


# ---------------------------------------------------------------------------
# Host-side edge preprocessing
# ---------------------------------------------------------------------------

def prep_edges(edge_index, n_nodes=N_NODES, ncores=NCORES):
    """Sort edges by dst, shard by dst range, pad each (core, block) segment
    to a common per-block tile count, and build the device index arrays.

    Returns (Tb, per_core) where Tb[b] is the number of 128-edge tiles of
    block b (shared by all cores) and per_core[c] is a dict with:
      idx : [128, TT*8] int16  wrapped dma_gather indices (pad -> n_nodes)
      oh  : [128, TT*128] bf16 one-hot; oh[e, g*128+j] = dst(g,e) == j
      ohT : [128, TT*128] bf16 transposed; ohT[j, g*128+e] = dst(g,e) == j
    """
    npc = n_nodes // ncores
    nb = (npc + P - 1) // P
    src = np.asarray(edge_index[0], dtype=np.int64)
    dst = np.asarray(edge_index[1], dtype=np.int64)
    order = np.argsort(dst, kind="stable")
    src_s, dst_s = src[order], dst[order]

    core_of = dst_s // npc
    blk_of = core_of * nb + (dst_s - core_of * npc) // P
    counts = np.bincount(blk_of, minlength=ncores * nb).reshape(ncores, nb)
    ends = np.cumsum(counts.reshape(-1)).reshape(ncores, nb)
    starts = ends - counts

    tiles = (counts + P - 1) // P            # [ncores, nb]
    Tb = np.maximum(tiles.max(axis=0), 1)    # shared per-block tile count
    TT = int(Tb.sum())
    offs = np.concatenate([[0], np.cumsum(Tb)[:-1]])  # tile offset per block

    jj = np.arange(P, dtype=np.int32)
    per_core = []
    for c in range(ncores):
        idx_flat = np.full(TT * P, n_nodes, dtype=np.int64)  # pad -> zero row
        dloc_flat = np.full(TT * P, -1, dtype=np.int32)      # pad -> -1
        for b in range(nb):
            s, e = starts[c, b], ends[c, b]
            o = offs[b] * P
            idx_flat[o:o + e - s] = src_s[s:e]
            dloc_flat[o:o + e - s] = (dst_s[s:e] - c * npc - b * P)
        # Wrap gather indices: within a gather call over a 128-aligned
        # sub-range, index j lives at [j % 16, j // 16], replicated to 128
        # partitions. The wrap is position-linear, so any 128-aligned
        # sub-range of the stream can be sliced out for one call.
        idx_w = np.tile(idx_flat.astype(np.int16).reshape(TT * 8, 16).T, (8, 1))
        dloc = dloc_flat.reshape(TT, P)      # [tile, edge lane]
        oh = (dloc[:, :, None] == jj[None, None, :])            # [g, e, j]
        oh_h = np.ascontiguousarray(
            oh.transpose(1, 0, 2).reshape(P, TT * P)).astype(BF16)
        ohT_h = np.ascontiguousarray(
            oh.transpose(2, 0, 1).reshape(P, TT * P)).astype(BF16)
        per_core.append({"idx": np.ascontiguousarray(idx_w),
                         "oh": oh_h, "ohT": ohT_h})
    return [int(t) for t in Tb], per_core


# ---------------------------------------------------------------------------
# Bass program builder (one GATv2 layer, optionally + heads)
# ---------------------------------------------------------------------------

def build_layer(nn, npc, in_dim, heads, Tb, elu, heads_out, num_devices=NCORES):
    import concourse.bacc as bacc
    import concourse.tile as tile
    import concourse.mybir as mybir
    from contextlib import ExitStack

    f32 = mybir.dt.float32
    bf = mybir.dt.bfloat16
    AF = mybir.ActivationFunctionType
    ALU = mybir.AluOpType
    AX = mybir.AxisListType

    C = 128
    H = heads
    F = H * C
    KC = in_dim // P
    NB = len(Tb)
    TT = sum(Tb)
    blk_rows = [min(P, npc - b * P) for b in range(NB)]
    n_node_tiles = (nn + P - 1) // P

    nc = bacc.Bacc("TRN2", target_bir_lowering=False, debug=False,
                   num_devices=num_devices, num_swdge_queues=NQ)

    # --- DRAM tensors ---
    xT = nc.dram_tensor("xT", [in_dim, nn], bf, kind="ExternalInput").ap()
    xoT = nc.dram_tensor("xoT", [in_dim, npc], bf, kind="ExternalInput").ap()
    wl = nc.dram_tensor("wl", [in_dim, F], bf, kind="ExternalInput").ap()
    wr = nc.dram_tensor("wr", [in_dim, F], bf, kind="ExternalInput").ap()
    bl1 = nc.dram_tensor("bl1", [1, F], bf, kind="ExternalInput").ap()
    br1 = nc.dram_tensor("br1", [1, F], bf, kind="ExternalInput").ap()
    ones_d = nc.dram_tensor("ones1", [1, P], bf, kind="ExternalInput").ap()
    attb = nc.dram_tensor("attb", [P, F], bf, kind="ExternalInput").ap()
    biasb = nc.dram_tensor("biasb", [P, F], bf, kind="ExternalInput").ap()
    idx_d = nc.dram_tensor("idx", [P, TT * 8], mybir.dt.int16,
                           kind="ExternalInput").ap()
    oh_d = nc.dram_tensor("oh", [P, TT * P], bf, kind="ExternalInput").ap()
    ohT_d = nc.dram_tensor("ohT", [P, TT * P], bf, kind="ExternalInput").ap()
    ident_d = nc.dram_tensor("ident", [P, P], bf, kind="ExternalInput").ap()
    if heads_out:
        headw = nc.dram_tensor("headw", [P, 2 * C], bf, kind="ExternalInput").ap()
        headb = nc.dram_tensor("headb", [P, 2], f32, kind="ExternalInput").ap()
        an_d = nc.dram_tensor("an", [npc, 1], f32, kind="ExternalOutput").ap()
        rc_d = nc.dram_tensor("rc", [npc, 1], f32, kind="ExternalOutput").ap()
    else:
        h_d = nc.dram_tensor("h_own", [npc, F], bf, kind="ExternalOutput").ap()
    xl_full = nc.dram_tensor("xl_full", [nn + 1, F], bf, kind="Internal").ap()

    with tile.TileContext(nc) as tc, ExitStack() as es:
        cp = es.enter_context(tc.tile_pool(name="const", bufs=1))

        wl_sb = cp.tile([P, KC, F], bf, tag="wl")
        nc.sync.dma_start(wl_sb[:], wl.rearrange("(c k) f -> k c f", k=P))
        wr_sb = cp.tile([P, KC, F], bf, tag="wr")
        nc.sync.dma_start(wr_sb[:], wr.rearrange("(c k) f -> k c f", k=P))
        bl1_sb = cp.tile([1, F], bf, tag="bl1")
        nc.sync.dma_start(bl1_sb[:], bl1[:])
        br1_sb = cp.tile([1, F], bf, tag="br1")
        nc.sync.dma_start(br1_sb[:], br1[:])
        ones_sb = cp.tile([1, P], bf, tag="ones")
        nc.sync.dma_start(ones_sb[:], ones_d[:])
        attb_sb = cp.tile([P, F], bf, tag="attb")
        nc.sync.dma_start(attb_sb[:], attb[:])
        biasb_sb = cp.tile([P, F], bf, tag="biasb")
        nc.sync.dma_start(biasb_sb[:], biasb[:])
        idx_sb = cp.tile([P, TT * 8], mybir.dt.int16, tag="idx")
        nc.sync.dma_start(idx_sb[:], idx_d[:])
        ident_sb = cp.tile([P, P], bf, tag="ident")
        nc.sync.dma_start(ident_sb[:], ident_d[:])
        if heads_out:
            headw_sb = cp.tile([P, 2 * C], bf, tag="headw")
            nc.sync.dma_start(headw_sb[:], headw[:])
            headb_sb = cp.tile([P, 2], f32, tag="headb")
            nc.sync.dma_start(headb_sb[:], headb[:])

        xr_sb = cp.tile([P, NB * F], bf, tag="xr")
        nc.vector.memset(xr_sb[:], 0.0)

        # ---- dense phase: xl_full = x @ wl + bl (all nodes), xr (own) ----
        with tc.tile_pool(name="dpsum", bufs=2, space="PSUM") as dps, \
             tc.tile_pool(name="dwork", bufs=3) as dw:
            xTr = xT.rearrange("(c k) f -> k c f", k=P)
            full_tiles = nn // P
            xlr = xl_full[0:full_tiles * P, :].rearrange(
                "(t p) f -> p t f", p=P)
            groups = [(m0, min(m0 + DB, full_tiles))
                      for m0 in range(0, full_tiles, DB)]
            for (m0, m1) in groups:
                db = m1 - m0
                lt = dw.tile([P, KC, DB * P], bf, tag="lt")
                nc.sync.dma_start(lt[:, :, :db * P],
                                  xTr[:, :, m0 * P:m1 * P])
                ot = dw.tile([P, DB, F], bf, tag="ot")
                for t in range(db):
                    ps = dps.tile([P, F], f32, tag="ps")
                    nc.tensor.matmul(ps[:], lhsT=ones_sb[:], rhs=bl1_sb[:],
                                     start=True, stop=False)
                    for c in range(KC):
                        nc.tensor.matmul(
                            ps[:], lhsT=lt[:, c, t * P:(t + 1) * P],
                            rhs=wl_sb[:, c, :],
                            start=False, stop=(c == KC - 1))
                    nc.scalar.copy(ot[:, t, :], ps[:])
                nc.sync.dma_start(xlr[:, m0:m1, :], ot[:, :db, :])
            # ragged tail tile + zero pad row
            r = nn - full_tiles * P
            if r:
                lt = dw.tile([P, KC, P], bf, tag="lt")
                nc.sync.dma_start(lt[:, :, :r],
                                  xTr[:, :, full_tiles * P:nn])
                ps = dps.tile([P, F], f32, tag="ps")
                nc.tensor.matmul(ps[:r], lhsT=ones_sb[:, :r], rhs=bl1_sb[:],
                                 start=True, stop=False)
                for c in range(KC):
                    nc.tensor.matmul(ps[:r], lhsT=lt[:, c, :r],
                                     rhs=wl_sb[:, c, :],
                                     start=False, stop=(c == KC - 1))
                ot = dw.tile([P, F], bf, tag="ot2")
                nc.scalar.copy(ot[:r], ps[:r])
                nc.sync.dma_start(xl_full[full_tiles * P:nn, :], ot[:r])
            zt = dw.tile([1, F], bf, tag="zt")
            nc.vector.memset(zt[:], 0.0)
            nc.sync.dma_start(xl_full[nn:nn + 1, :], zt[:])

            xoTr = xoT.rearrange("(c k) f -> k c f", k=P)
            bgroups = [(b0, min(b0 + DB, NB)) for b0 in range(0, NB, DB)]
            for (b0, b1) in bgroups:
                w = blk_rows[b1 - 1] + (b1 - 1 - b0) * P
                lt = dw.tile([P, KC, DB * P], bf, tag="lt")
                nc.sync.dma_start(lt[:, :, :w], xoTr[:, :, b0 * P:b0 * P + w])
                for b in range(b0, b1):
                    r = blk_rows[b]
                    o = (b - b0) * P
                    ps = dps.tile([P, F], f32, tag="ps")
                    nc.tensor.matmul(ps[:r], lhsT=ones_sb[:, :r], rhs=br1_sb[:],
                                     start=True, stop=False)
                    for c in range(KC):
                        nc.tensor.matmul(ps[:r], lhsT=lt[:, c, o:o + r],
                                         rhs=wr_sb[:, c, :],
                                         start=False, stop=(c == KC - 1))
                    nc.scalar.copy(xr_sb[:r, b * F:(b + 1) * F], ps[:r])

        tc.strict_bb_all_engine_barrier()

        # ---- edge phase (2-tile software pipeline over the tile stream) ----
        stream = []  # (block, global tile, first, last)
        g = 0
        for b in range(NB):
            for t in range(Tb[b]):
                stream.append((b, g, t == 0, t == Tb[b] - 1))
                g += 1
        Tt = len(stream)
        chunks = [(c0, min(c0 + GCH, Tt)) for c0 in range(0, Tt, GCH)]
        chunk_of = {}
        for ci, (c0, c1) in enumerate(chunks):
            for gg in range(c0, c1):
                chunk_of[gg] = ci

        gsems = [nc.alloc_semaphore(f"gsem{q}") for q in range(NQ)]

        with tc.tile_pool(name="gat", bufs=3) as gp, \
             tc.tile_pool(name="ohc", bufs=3) as ohcp, \
             tc.tile_pool(name="elp", bufs=3) as elp, \
             tc.tile_pool(name="msp", bufs=3) as msp, \
             tc.tile_pool(name="scp", bufs=4) as scp, \
             tc.tile_pool(name="blk", bufs=2) as blk, \
             tc.tile_pool(name="ps_s", bufs=3, space="PSUM") as psp, \
             tc.tile_pool(name="ps_out", bufs=2, space="PSUM") as ps_out_p, \
             tc.tile_pool(name="ps_den", bufs=2, space="PSUM") as ps_den_p:

            xg_tiles = {}      # chunk idx -> (tile, c0)
            qseq = [0] * NQ    # completed-gather count per queue
            chunk_sem = {}     # chunk idx -> (queue, wait value)
            waited = set()     # (engine, chunk) wait already emitted
            ohc_tiles = {}     # chunk idx -> (oh tile, ohT tile, c0)
            el_t, exb_t, ps_s_t = {}, {}, {}
            pout_b, pden_b = {}, {}

            def emit_chunk(ci):
                c0, c1 = chunks[ci]
                n_t = c1 - c0
                q = ci % NQ
                xg = gp.tile([P, GCH, F], bf, tag="xg")
                if PREP_GATHER:
                    nc.gpsimd.dma_gather(
                        xg[:, :n_t, :], xl_full[:],
                        idx_sb[:, c0 * 8:c1 * 8],
                        num_idxs=n_t * P, num_idxs_reg=n_t * P, elem_size=F,
                        single_packet=False, queue_num=q,
                        prepare_only=True, sem=gsems[q])
                    nc.gpsimd.trigger_dma(count=None, queue_num=q)
                    qseq[q] += 1
                    chunk_sem[ci] = (q, 16 * qseq[q])
                else:
                    nc.gpsimd.dma_gather(
                        xg[:, :n_t, :], xl_full[:],
                        idx_sb[:, c0 * 8:c1 * 8],
                        num_idxs=n_t * P, num_idxs_reg=n_t * P, elem_size=F,
                        single_packet=False, queue_num=q)
                xg_tiles[ci] = (xg, c0)
                ohc = ohcp.tile([P, GCH * P], bf, tag="ohc")
                nc.sync.dma_start(ohc[:, :n_t * P], oh_d[:, c0 * P:c1 * P])
                ohTc = ohcp.tile([P, GCH * P], bf, tag="ohTc")
                nc.sync.dma_start(ohTc[:, :n_t * P], ohT_d[:, c0 * P:c1 * P])
                ohc_tiles[ci] = (ohc, ohTc, c0)

            def xg_slice(gg):
                xg, c0 = xg_tiles[chunk_of[gg]]
                return xg[:, gg - c0, :]

            def oh_slices(gg):
                ohc, ohTc, c0 = ohc_tiles[chunk_of[gg]]
                o = (gg - c0) * P
                return ohc[:, o:o + P], ohTc[:, o:o + P]

            def block_end(b):
                r = blk_rows[b]
                den = blk.tile([P, H], f32, tag="den")
                nc.vector.tensor_scalar(out=den[:], in0=pden_b[b][:],
                                        scalar1=1e-16, scalar2=None,
                                        op0=ALU.add)
                rec = blk.tile([P, H], f32, tag="rec")
                nc.vector.reciprocal(rec[:], den[:])
                hb = blk.tile([P, F], bf, tag="hb")
                for h in range(H):
                    nc.scalar.activation(hb[:, h * C:(h + 1) * C],
                                         pout_b[b][:, h * C:(h + 1) * C],
                                         AF.Copy, scale=rec[:, h:h + 1])
                hc = blk.tile([P, F], bf, tag="hc")
                nc.vector.tensor_tensor(out=hc[:], in0=hb[:], in1=biasb_sb[:],
                                        op=ALU.add)
                if elu:
                    # h+1 is stored; host folds the -1 into layer-2 biases.
                    mn = blk.tile([P, F], bf, tag="mn")
                    nc.vector.tensor_scalar_min(mn[:], hc[:], 0.0)
                    en = blk.tile([P, F], bf, tag="en")
                    nc.scalar.activation(en[:], mn[:], AF.Exp)
                    mx = blk.tile([P, F], bf, tag="mx")
                    nc.vector.tensor_scalar_max(mx[:], hc[:], 0.0)
                    ho = blk.tile([P, F], bf, tag="ho")
                    nc.vector.tensor_tensor(out=ho[:], in0=mx[:], in1=en[:],
                                            op=ALU.add)
                    nc.sync.dma_start(h_d[b * P:b * P + r, :], ho[:r])
                elif heads_out:
                    for j, outd in enumerate([an_d, rc_d]):
                        tmp = blk.tile([P, C], bf, tag=f"ht{j}")
                        nc.vector.tensor_tensor(
                            out=tmp[:], in0=hc[:],
                            in1=headw_sb[:, j * C:(j + 1) * C], op=ALU.mult)
                        red = blk.tile([P, 1], f32, tag=f"hr{j}")
                        nc.vector.tensor_reduce(out=red[:], in_=tmp[:],
                                                axis=AX.X, op=ALU.add)
                        sg = blk.tile([P, 1], f32, tag=f"hs{j}")
                        nc.scalar.activation(sg[:], red[:], AF.Sigmoid,
                                             bias=headb_sb[:, j:j + 1])
                        nc.sync.dma_start(outd[b * P:b * P + r, :], sg[:r])
                else:
                    nc.sync.dma_start(h_d[b * P:b * P + r, :], hc[:r])
                del pout_b[b], pden_b[b]

            emit_chunk(0)
            if len(chunks) > 1:
                emit_chunk(1)

            for i in range(Tt + 2):
                if i < Tt:
                    b, gg, first, last = stream[i]
                    ci = chunk_of[gg]
                    if gg == chunks[ci][0] and ci >= 1 and ci + 1 < len(chunks):
                        emit_chunk(ci + 1)
                    # S: s = xr[dst] + xl[src] accumulated in PSUM
                    if PREP_GATHER and ("pe", ci) not in waited:
                        qn, val = chunk_sem[ci]
                        nc.tensor.wait_ge(gsems[qn], val)
                        waited.add(("pe", ci))
                    _, ohT_s = oh_slices(gg)
                    ps_s = psp.tile([P, F], f32, tag="s")
                    nc.tensor.matmul(ps_s[:], lhsT=ohT_s,
                                     rhs=xr_sb[:, b * F:(b + 1) * F],
                                     start=True, stop=False)
                    nc.tensor.matmul(ps_s[:], lhsT=ident_sb[:],
                                     rhs=xg_slice(gg), start=False, stop=True)
                    ps_s_t[gg] = ps_s

                j = i - 1
                if 0 <= j < Tt:
                    gj = stream[j][1]
                    # D2: att dot -> per-head scores
                    pr = msp.tile([P, F], bf, tag="pr")
                    nc.vector.tensor_tensor(out=pr[:], in0=el_t[gj],
                                            in1=attb_sb[:], op=ALU.mult)
                    sc = scp.tile([P, H], bf, tag="sc")
                    with nc.allow_low_precision(reason="bf16 scores validated"):
                        nc.vector.tensor_reduce(
                            out=sc[:],
                            in_=pr[:].rearrange("p (h c) -> p h c", c=C),
                            axis=AX.X, op=ALU.add)
                    # SC2: exp (bf16; feeds both msg scaling and pden rhs)
                    exb = scp.tile([P, H], bf, tag="exb")
                    nc.scalar.activation(exb[:], sc[:], AF.Exp)
                    exb_t[gj] = exb

                if i < Tt:
                    # SC1: leaky relu (PSUM -> bf16)
                    gi = stream[i][1]
                    el = elp.tile([P, F], bf, tag="el")
                    nc.scalar.activation(el[:], ps_s_t[gi][:], AF.Prelu,
                                         alpha=NEG_SLOPE)
                    el_t[gi] = el
                    del ps_s_t[gi]

                k = i - 2
                if 0 <= k:
                    bk, gk, first, last = stream[k]
                    # D3: per-head message scaling
                    ck = chunk_of[gk]
                    if PREP_GATHER and ("dve", ck) not in waited:
                        qn, val = chunk_sem[ck]
                        nc.vector.wait_ge(gsems[qn], val)
                        waited.add(("dve", ck))
                    msg = msp.tile([P, F], bf, tag="msg")
                    exb = exb_t[gk]
                    exv = exb[:].rearrange("p (h o) -> p h o", o=1)
                    nc.vector.tensor_tensor(
                        out=msg[:].rearrange("p (h c) -> p h c", c=C),
                        in0=xg_slice(gk).rearrange("p (h c) -> p h c", c=C),
                        in1=exv.to_broadcast([P, H, C]), op=ALU.mult)
                    # B: segment sums
                    oh_s, _ = oh_slices(gk)
                    if first:
                        pout_b[bk] = ps_out_p.tile([P, F], f32, tag="pout",
                                                   name=f"pout{bk}")
                        pden_b[bk] = ps_den_p.tile([P, H], f32, tag="pden",
                                                   name=f"pden{bk}")
                    nc.tensor.matmul(pden_b[bk][:], lhsT=oh_s, rhs=exb[:],
                                     start=first, stop=last,
                                     skip_group_check=True)
                    nc.tensor.matmul(pout_b[bk][:], lhsT=oh_s, rhs=msg[:],
                                     start=first, stop=last,
                                     skip_group_check=True)
                    del exb_t[gk]
                    el_t.pop(gk, None)
                    if last:
                        block_end(bk)

    nc.compile()
    return nc


# ---------------------------------------------------------------------------
# Host orchestration
# ---------------------------------------------------------------------------

def _rep(v):
    """Replicate a 1-D param vector across 128 partitions (bf16)."""
    v = np.asarray(v, dtype=np.float32).reshape(-1)
    return np.tile(v[None, :], (P, 1)).astype(BF16)


TRACE = False          # set by test harness to capture NTFF profiles
TRACE_LAUNCHES = (0, 1)  # which launches to trace when TRACE is on
LAST_RESULTS = []      # BassKernelResults of the last kernel() call


def run_spmd(nc, in_maps, trace=False, trace_kwargs=None):
    from concourse import bass_utils
    launch = len(LAST_RESULTS)
    do_trace = (trace or TRACE) and launch in TRACE_LAUNCHES
    res = bass_utils.run_bass_kernel_spmd(
        nc, in_maps, core_ids=list(range(len(in_maps))), trace=do_trace,
        **(trace_kwargs or {}))
    LAST_RESULTS.append(res)
    return res


def _bf(a):
    return np.ascontiguousarray(np.asarray(a).astype(BF16))


def kernel(x, edge_index, W1l, b1l, W1r, b1r, att1, bias1,
           W2l, b2l, W2r, b2r, att2, bias2, Wa, ba, Wrc, brc):
    x = np.asarray(x, dtype=np.float32)
    Tb, per_core = prep_edges(edge_index)

    ident = np.eye(P, dtype=np.float32).astype(BF16)
    ones1 = np.ones((1, P), dtype=np.float32).astype(BF16)
    consts = {"ident": ident, "ones1": ones1}

    nc1 = _get_program(1, tuple(Tb))
    xb = x.astype(BF16)
    common1 = {
        "xT": np.ascontiguousarray(xb.T),
        "wl": _bf(W1l), "wr": _bf(W1r),
        "bl1": _bf(np.asarray(b1l, np.float32).reshape(1, -1)),
        "br1": _bf(np.asarray(b1r, np.float32).reshape(1, -1)),
        "attb": _rep(np.asarray(att1, np.float32).reshape(-1)),
        "biasb": _rep(bias1),
        **consts,
    }
    in_maps = []
    for c in range(NCORES):
        m = dict(common1)
        m["xoT"] = np.ascontiguousarray(xb[c * NPC:(c + 1) * NPC].T)
        m["idx"] = per_core[c]["idx"]
        m["oh"] = per_core[c]["oh"]
        m["ohT"] = per_core[c]["ohT"]
        in_maps.append(m)
    LAST_RESULTS.clear()
    res1 = run_spmd(nc1, in_maps)
    h = np.concatenate([np.asarray(res1.results[c]["h_own"])
                        for c in range(NCORES)], axis=0)  # = elu(h_true)+1

    nc2 = _get_program(2, tuple(Tb))
    # h_true = h - 1: fold the -1 into the layer-2 linear biases.
    b2l_eff = np.asarray(b2l, np.float32) - np.asarray(W2l, np.float32).sum(0)
    b2r_eff = np.asarray(b2r, np.float32) - np.asarray(W2r, np.float32).sum(0)
    common2 = {
        "xT": np.ascontiguousarray(h.T),
        "wl": _bf(W2l), "wr": _bf(W2r),
        "bl1": _bf(b2l_eff.reshape(1, -1)),
        "br1": _bf(b2r_eff.reshape(1, -1)),
        "attb": _rep(np.asarray(att2, np.float32).reshape(-1)),
        "biasb": _rep(bias2),
        "headw": np.concatenate(
            [_rep(np.asarray(Wa, np.float32).reshape(-1)),
             _rep(np.asarray(Wrc, np.float32).reshape(-1))], axis=1),
        "headb": np.concatenate(
            [np.full((P, 1), np.float32(np.asarray(ba).reshape(())), np.float32),
             np.full((P, 1), np.float32(np.asarray(brc).reshape(())), np.float32)],
            axis=1),
        **consts,
    }
    in_maps2 = []
    for c in range(NCORES):
        m = dict(common2)
        m["xoT"] = np.ascontiguousarray(h[c * NPC:(c + 1) * NPC].T)
        m["idx"] = per_core[c]["idx"]
        m["oh"] = per_core[c]["oh"]
        m["ohT"] = per_core[c]["ohT"]
        in_maps2.append(m)
    res2 = run_spmd(nc2, in_maps2)
    an = np.concatenate([np.asarray(res2.results[c]["an"])
                         for c in range(NCORES)], axis=0)
    rc = np.concatenate([np.asarray(res2.results[c]["rc"])
                         for c in range(NCORES)], axis=0)
    return an, rc


_PROGRAMS = {}


def _get_program(layer, tb_key):
    key = (layer, tb_key)
    if key not in _PROGRAMS:
        if layer == 1:
            _PROGRAMS[key] = build_layer(N_NODES, NPC, IN_DIM, HEADS,
                                         list(tb_key), elu=True, heads_out=False)
        else:
            _PROGRAMS[key] = build_layer(N_NODES, NPC, HEADS * HID, 1,
                                         list(tb_key), elu=False, heads_out=True)
    return _PROGRAMS[key]
